# revision 1
# baseline (speedup 1.0000x reference)
"""ConvergedInhibition forward on 8 Trainium2 NeuronCores.

The reference computes, independently for every (n, h, w) pixel, a
frequency-domain deconvolution along the channel axis C=128:

    out = ifft(fft(x, axis=C) / Fk).real

Division by Fk in frequency space is circular convolution with
g = ifft(1/Fk) (real, since delta-k is real), i.e. a fixed 128x128
circulant matrix M applied to every channel vector:

    out[n, :, h, w] = M @ x[n, :, h, w],   M[c, c'] = g[(c - c') mod C]

So the heavy work is a tiny stationary matmul swept over a 134 MB
activation tensor -> memory-bound tensor-engine kernel. The length-128
filter preprocessing (FFT of a 128-vector) is negligible and done on
host in float64.

Sharding: data-parallel over batch N=64 -> 8 batches per core, no
cross-core communication. Each core streams (128, 2048) 1 MB half-tiles:
HWDGE DMA in on the sync queue, matmul against the stationary
inverse-circulant lhsT in 512-col PSUM-bank chunks, drain PSUM->SBUF on
both copy engines, DMA out on the scalar engine's HWDGE queue (so
pending outputs never head-of-line block input loads). The first and
last batch taper to quarter-tiles for fast pipeline fill/drain, and
input lookahead is capped at 4 tiles so every core presents steady
mixed read+write HBM traffic (a read burst followed by a write-only
tail loses ~10 us to paired-core contention). Measured on HW: 93-95 us
per core vs a ~94 us HBM roofline (33.6 MB/core at 358 GB/s).
"""

import numpy as np

import concourse.bass as bass
import concourse.mybir as mybir
from concourse import bacc
from concourse.bass_utils import run_bass_kernel_spmd
from concourse.tile import TileContext

N_CORES = 8
PSUM_CHUNK = 512  # fp32 elements per PSUM bank


def _inverse_circulant_lhsT(filt: np.ndarray, C: int) -> np.ndarray:
    """Build the stationary matmul operand lhsT (K x M layout).

    out[m] = sum_k M[m, k] x[k] with M[m, k] = g[(m - k) mod C], and the
    tensor engine computes lhsT.T @ rhs, so lhsT[k, m] = g[(m - k) mod C].
    """
    scope = filt.shape[-1]
    pad_left = (C - scope) // 2
    k = np.zeros(C, dtype=np.float64)
    k[pad_left : pad_left + scope] = filt.reshape(-1).astype(np.float64)
    k = np.roll(k, C // 2 + 1)
    delta = np.zeros(C, dtype=np.float64)
    delta[0] = 1.0
    g = np.fft.ifft(1.0 / np.fft.fft(delta - k)).real
    j = np.arange(C)
    return g[(j[None, :] - j[:, None]) % C].astype(np.float32)


def build_nc(
    b_per_core: int, C: int, P: int, use_f32r: bool = False, half: int = 2048
) -> bacc.Bacc:
    # float32r streams fp32 bits through the PE in a single reduced-mantissa
    # pass (1 cycle/row at N>=512) instead of fp32's two half-speed passes.
    # Measured: no e2e gain (DMA-paced kernel) and ~1e-4 rel err, so fp32
    # stays the default.
    mm_dt = mybir.dt.float32r if use_f32r else mybir.dt.float32
    nc = bacc.Bacc("TRN2", target_bir_lowering=False, debug=False)
    x = nc.dram_tensor("x", [b_per_core, C, P], mm_dt, kind="ExternalInput")
    w = nc.dram_tensor("w", [C, C], mm_dt, kind="ExternalInput")
    y = nc.dram_tensor("y", [b_per_core, C, P], mybir.dt.float32, kind="ExternalOutput")

    # 1 MB sub-tiles throughout. A tapered fill/drain and tighter input
    # lookahead were both measured (~1 us WORSE in the fast mode, no effect
    # on the contention-driven slow mode), so uniform tiles stay.
    def batch_widths(b):
        return [half] * (P // half)

    with TileContext(nc) as tc:
        with (
            tc.tile_pool(name="wp", bufs=1) as wp,
            tc.tile_pool(name="xp", bufs=8) as xp,
            tc.tile_pool(name="yp", bufs=8) as yp,
            tc.tile_pool(name="pp", bufs=8, space="PSUM") as pp,
        ):
            wt = wp.tile([C, C], mm_dt)
            nc.sync.dma_start(wt[:], w[:, :])
            for b in range(b_per_core):
                off = 0
                for width in batch_widths(b):
                    xt = xp.tile([C, width], mm_dt, tag="x")
                    nc.sync.dma_start(xt[:], x[b, :, bass.ds(off, width)])
                    yt = yp.tile([C, width], mybir.dt.float32, tag="y")
                    n_chunks = (width + PSUM_CHUNK - 1) // PSUM_CHUNK
                    for j in range(n_chunks):
                        cw = min(PSUM_CHUNK, width - j * PSUM_CHUNK)
                        pt = pp.tile([C, cw], mybir.dt.float32)
                        cols = bass.ds(j * PSUM_CHUNK, cw)
                        nc.tensor.matmul(
                            pt[:], wt[:], xt[:, cols], start=True, stop=True
                        )
                        # PSUM has no DMA route: drain via both copy engines —
                        # early chunks on DVE, late on ACT, so the ACT-queue
                        # out-DMA below follows its inputs mostly in program
                        # order instead of a cross-engine wait.
                        if j < n_chunks / 2:
                            nc.vector.tensor_copy(yt[:, cols], pt[:])
                        else:
                            nc.scalar.copy(yt[:, cols], pt[:])
                    # Out-DMAs ride the scalar engine's own HWDGE queue so a
                    # pending output never head-of-line blocks input loads on
                    # the sync queue.
                    nc.scalar.dma_start(y[b, :, bass.ds(off, width)], yt[:])
                    off += width
    nc.compile()
    return nc


_NC_CACHE: dict = {}


def _run(activations, inhibition_filter, use_f32r=False, **spmd_kwargs):
    act = np.ascontiguousarray(np.asarray(activations, dtype=np.float32))
    filt = np.asarray(inhibition_filter, dtype=np.float32)
    B, C, H, W = act.shape
    P = H * W
    assert B % N_CORES == 0
    b_per_core = B // N_CORES

    lhsT = _inverse_circulant_lhsT(filt, C)
    key = (b_per_core, C, P, use_f32r)
    nc = _NC_CACHE.get(key)
    if nc is None:
        nc = _NC_CACHE[key] = build_nc(b_per_core, C, P, use_f32r=use_f32r)

    xs = act.reshape(N_CORES, b_per_core, C, P)
    in_maps = [{"x": xs[i], "w": lhsT} for i in range(N_CORES)]
    res = run_bass_kernel_spmd(nc, in_maps, core_ids=list(range(N_CORES)), **spmd_kwargs)
    out = np.stack([res.results[i]["y"] for i in range(N_CORES)], axis=0)
    return out.reshape(B, C, H, W), res


def kernel(activations: np.ndarray, inhibition_filter: np.ndarray) -> np.ndarray:
    out, _ = _run(activations, inhibition_filter)
    return out



# revision 5
# speedup vs baseline: 1.4710x; 1.4710x over previous
"""ConvergedInhibition forward on 8 Trainium2 NeuronCores.

The reference computes, independently for every (n, h, w) pixel, a
frequency-domain deconvolution along the channel axis C=128:

    out = ifft(fft(x, axis=C) / Fk).real

Division by Fk in frequency space is circular convolution with
g = ifft(1/Fk) (real, since delta-k is real), i.e. a fixed 128x128
circulant matrix M applied to every channel vector:

    out[n, :, h, w] = M @ x[n, :, h, w],   M[c, c'] = g[(c - c') mod C]

So the heavy work is a tiny stationary matmul swept over a 134 MB
activation tensor -> memory-bound tensor-engine kernel. The length-128
filter preprocessing (FFT of a 128-vector) is negligible and done on
host in float64.

Sharding: data-parallel over batch N=64 -> 8 batches per core, no
cross-core communication. Each core streams (128, 2048) 1 MB half-tiles:
HWDGE DMA in on the sync queue, matmul against the stationary
inverse-circulant lhsT in 512-col PSUM-bank chunks, drain PSUM->SBUF on
both copy engines, DMA out on the scalar engine's HWDGE queue (so
pending outputs never head-of-line block input loads). The first and
last batch taper to quarter-tiles for fast pipeline fill/drain, and
input lookahead is capped at 4 tiles so every core presents steady
mixed read+write HBM traffic (a read burst followed by a write-only
tail loses ~10 us to paired-core contention). Measured on HW: 93-95 us
per core vs a ~94 us HBM roofline (33.6 MB/core at 358 GB/s).
"""

import ml_dtypes
import numpy as np

import concourse.bass as bass
import concourse.mybir as mybir
from concourse import bacc
from concourse.bass_utils import run_bass_kernel_spmd
from concourse.tile import TileContext

N_CORES = 8
PSUM_CHUNK = 512  # fp32 elements per PSUM bank


def _inverse_circulant_lhsT(filt: np.ndarray, C: int) -> np.ndarray:
    """Build the stationary matmul operand lhsT (K x M layout).

    out[m] = sum_k M[m, k] x[k] with M[m, k] = g[(m - k) mod C], and the
    tensor engine computes lhsT.T @ rhs, so lhsT[k, m] = g[(m - k) mod C].
    """
    scope = filt.shape[-1]
    pad_left = (C - scope) // 2
    k = np.zeros(C, dtype=np.float64)
    k[pad_left : pad_left + scope] = filt.reshape(-1).astype(np.float64)
    k = np.roll(k, C // 2 + 1)
    delta = np.zeros(C, dtype=np.float64)
    delta[0] = 1.0
    g = np.fft.ifft(1.0 / np.fft.fft(delta - k)).real
    j = np.arange(C)
    return g[(j[None, :] - j[:, None]) % C].astype(np.float32)


def build_nc(
    b_per_core: int, C: int, P: int, io: str = "bf16", half: int = 2048
) -> bacc.Bacc:
    # io="bf16": stream activations in AND out as bfloat16 — halves HBM
    # traffic (the binding constraint: fp32 was measured at the 358 GB/s
    # roofline) and runs the PE at 1 cycle/col instead of fp32's 4. The
    # 2e-2 rel-err gate dwarfs bf16's ~2e-3 quantization error.
    # io="f32r" streams fp32 bits through the PE in a single reduced-
    # mantissa pass; measured: no e2e gain over f32 (DMA-paced kernel).
    mm_dt = {
        "bf16": mybir.dt.bfloat16,
        "f32": mybir.dt.float32,
        "f32r": mybir.dt.float32r,
    }[io]
    out_dt = mybir.dt.bfloat16 if io == "bf16" else mybir.dt.float32
    nc = bacc.Bacc("TRN2", target_bir_lowering=False, debug=False)
    x = nc.dram_tensor("x", [b_per_core, C, P], mm_dt, kind="ExternalInput")
    w = nc.dram_tensor("w", [C, C], mm_dt, kind="ExternalInput")
    y = nc.dram_tensor("y", [b_per_core, C, P], out_dt, kind="ExternalOutput")

    # 1 MB sub-tiles throughout. A tapered fill/drain and tighter input
    # lookahead were both measured (~1 us WORSE in the fast mode, no effect
    # on the contention-driven slow mode), so uniform tiles stay.
    def batch_widths(b):
        return [half] * (P // half)

    with TileContext(nc) as tc:
        with (
            tc.tile_pool(name="wp", bufs=1) as wp,
            tc.tile_pool(name="xp", bufs=8) as xp,
            tc.tile_pool(name="yp", bufs=8) as yp,
            tc.tile_pool(name="pp", bufs=8, space="PSUM") as pp,
        ):
            wt = wp.tile([C, C], mm_dt)
            nc.sync.dma_start(wt[:], w[:, :])
            for b in range(b_per_core):
                off = 0
                for width in batch_widths(b):
                    xt = xp.tile([C, width], mm_dt, tag="x")
                    nc.sync.dma_start(xt[:], x[b, :, bass.ds(off, width)])
                    yt = yp.tile([C, width], out_dt, tag="y")
                    n_chunks = (width + PSUM_CHUNK - 1) // PSUM_CHUNK
                    for j in range(n_chunks):
                        cw = min(PSUM_CHUNK, width - j * PSUM_CHUNK)
                        pt = pp.tile([C, cw], mybir.dt.float32)
                        cols = bass.ds(j * PSUM_CHUNK, cw)
                        nc.tensor.matmul(
                            pt[:], wt[:], xt[:, cols], start=True, stop=True
                        )
                        # PSUM has no DMA route: drain via both copy engines —
                        # early chunks on DVE, late on ACT, so the ACT-queue
                        # out-DMA below follows its inputs mostly in program
                        # order instead of a cross-engine wait.
                        if j < n_chunks / 2:
                            nc.vector.tensor_copy(yt[:, cols], pt[:])
                        else:
                            nc.scalar.copy(yt[:, cols], pt[:])
                    # Out-DMAs ride the scalar engine's own HWDGE queue so a
                    # pending output never head-of-line blocks input loads on
                    # the sync queue.
                    nc.scalar.dma_start(y[b, :, bass.ds(off, width)], yt[:])
                    off += width
    nc.compile()
    return nc


_NC_CACHE: dict = {}


def _run(activations, inhibition_filter, use_f32r=False, io=None, **spmd_kwargs):
    act = np.ascontiguousarray(np.asarray(activations, dtype=np.float32))
    filt = np.asarray(inhibition_filter, dtype=np.float32)
    B, C, H, W = act.shape
    P = H * W
    assert B % N_CORES == 0
    b_per_core = B // N_CORES
    if io is None:
        io = "f32r" if use_f32r else "bf16"

    lhsT = _inverse_circulant_lhsT(filt, C)
    key = (b_per_core, C, P, io)
    nc = _NC_CACHE.get(key)
    if nc is None:
        nc = _NC_CACHE[key] = build_nc(b_per_core, C, P, io=io)

    if io == "bf16":
        act = act.astype(ml_dtypes.bfloat16)
        lhsT = lhsT.astype(ml_dtypes.bfloat16)
    xs = act.reshape(N_CORES, b_per_core, C, P)
    in_maps = [{"x": xs[i], "w": lhsT} for i in range(N_CORES)]
    res = run_bass_kernel_spmd(nc, in_maps, core_ids=list(range(N_CORES)), **spmd_kwargs)
    out = np.stack([res.results[i]["y"] for i in range(N_CORES)], axis=0)
    return out.reshape(B, C, H, W).astype(np.float32), res


def kernel(activations: np.ndarray, inhibition_filter: np.ndarray) -> np.ndarray:
    out, _ = _run(activations, inhibition_filter)
    return out



# revision 6
# speedup vs baseline: 1.4818x; 1.0073x over previous
"""ConvergedInhibition forward on 8 Trainium2 NeuronCores.

The reference computes, independently for every (n, h, w) pixel, a
frequency-domain deconvolution along the channel axis C=128:

    out = ifft(fft(x, axis=C) / Fk).real

Division by Fk in frequency space is circular convolution with
g = ifft(1/Fk) (real, since delta-k is real), i.e. a fixed 128x128
circulant matrix M applied to every channel vector:

    out[n, :, h, w] = M @ x[n, :, h, w],   M[c, c'] = g[(c - c') mod C]

So the heavy work is a tiny stationary matmul swept over a 134 MB
activation tensor -> memory-bound tensor-engine kernel. The length-128
filter preprocessing (FFT of a 128-vector) is negligible and done on
host in float64.

Sharding: data-parallel over batch N=64 -> 8 batches per core, no
cross-core communication. Each core streams (128, 2048) 1 MB half-tiles:
HWDGE DMA in on the sync queue, matmul against the stationary
inverse-circulant lhsT in 512-col PSUM-bank chunks, drain PSUM->SBUF on
both copy engines, DMA out on the scalar engine's HWDGE queue (so
pending outputs never head-of-line block input loads). The first and
last batch taper to quarter-tiles for fast pipeline fill/drain, and
input lookahead is capped at 4 tiles so every core presents steady
mixed read+write HBM traffic (a read burst followed by a write-only
tail loses ~10 us to paired-core contention). Measured on HW: 93-95 us
per core vs a ~94 us HBM roofline (33.6 MB/core at 358 GB/s).
"""

import ml_dtypes
import numpy as np

import concourse.bass as bass
import concourse.mybir as mybir
from concourse import bacc
from concourse.bass_utils import run_bass_kernel_spmd
from concourse.tile import TileContext

N_CORES = 8
PSUM_CHUNK = 512  # fp32 elements per PSUM bank


def _inverse_circulant_lhsT(filt: np.ndarray, C: int) -> np.ndarray:
    """Build the stationary matmul operand lhsT (K x M layout).

    out[m] = sum_k M[m, k] x[k] with M[m, k] = g[(m - k) mod C], and the
    tensor engine computes lhsT.T @ rhs, so lhsT[k, m] = g[(m - k) mod C].
    """
    scope = filt.shape[-1]
    pad_left = (C - scope) // 2
    k = np.zeros(C, dtype=np.float64)
    k[pad_left : pad_left + scope] = filt.reshape(-1).astype(np.float64)
    k = np.roll(k, C // 2 + 1)
    delta = np.zeros(C, dtype=np.float64)
    delta[0] = 1.0
    g = np.fft.ifft(1.0 / np.fft.fft(delta - k)).real
    j = np.arange(C)
    return g[(j[None, :] - j[:, None]) % C].astype(np.float32)


def build_nc(b_per_core: int, C: int, P: int, io: str = "bf16") -> bacc.Bacc:
    # io="bf16": stream activations in AND out as bfloat16 — halves HBM
    # traffic (the binding constraint: fp32 was measured at the 358 GB/s
    # roofline) and runs the PE at 1 cycle/col instead of fp32's 4. The
    # 2e-2 rel-err gate dwarfs bf16's ~2e-3 quantization error.
    # io="f32r" streams fp32 bits through the PE in a single reduced-
    # mantissa pass; measured: no e2e gain over f32 (DMA-paced kernel).
    mm_dt = {
        "bf16": mybir.dt.bfloat16,
        "f32": mybir.dt.float32,
        "f32r": mybir.dt.float32r,
    }[io]
    out_dt = mybir.dt.bfloat16 if io == "bf16" else mybir.dt.float32
    nc = bacc.Bacc("TRN2", target_bir_lowering=False, debug=False)
    x = nc.dram_tensor("x", [b_per_core, C, P], mm_dt, kind="ExternalInput")
    w = nc.dram_tensor("w", [C, C], mm_dt, kind="ExternalInput")
    y = nc.dram_tensor("y", [b_per_core, C, P], out_dt, kind="ExternalOutput")

    n_ch = P // PSUM_CHUNK  # PSUM-bank-sized matmul chunks per batch row
    with TileContext(nc) as tc:
        with (
            tc.tile_pool(name="wp", bufs=1) as wp,
            tc.tile_pool(name="xp", bufs=1) as xp,
            tc.tile_pool(name="yp", bufs=b_per_core) as yp,
            tc.tile_pool(name="pp", bufs=8, space="PSUM") as pp,
        ):
            # All x/y tiles stay resident in SBUF (~16 MB of 24) so no
            # buffer-reuse edge ever throttles the pipeline; the tail was
            # measured pacing at 3.7 us/tile on the out-DMA -> y-reuse ->
            # drain loop with 8-deep rings.
            wt = wp.tile([C, C], mm_dt)
            nc.sync.dma_start(wt[:], w[:, :])
            # Batch 0 arrives in tapered pieces so the first matmul only
            # waits on a 128 KB transfer, not a 1 MB one; later batches are
            # single 1 MB loads (fewer descriptor-gens + semaphores).
            taper = [PSUM_CHUNK, PSUM_CHUNK, 2 * PSUM_CHUNK, P - 4 * PSUM_CHUNK]
            batches = []
            pieces = []
            off = 0
            for i, pw in enumerate(taper):
                t = xp.tile([C, pw], mm_dt, tag=f"x0_{i}", bufs=1)
                nc.sync.dma_start(t[:], x[0, :, bass.ds(off, pw)])
                pieces.append((t, off, pw))
                off += pw
            batches.append(pieces)
            for b in range(1, b_per_core):
                t = xp.tile([C, P], mm_dt, tag="xbig", bufs=b_per_core - 1)
                nc.sync.dma_start(t[:], x[b, :, :])
                batches.append([(t, 0, P)])
            for b in range(b_per_core):
                yt = yp.tile([C, P], out_dt, tag="y")
                for j in range(n_ch):
                    col0 = j * PSUM_CHUNK
                    xt, poff, pw = next(
                        p for p in batches[b] if p[1] <= col0 < p[1] + p[2]
                    )
                    pt = pp.tile([C, PSUM_CHUNK], mybir.dt.float32)
                    nc.tensor.matmul(
                        pt[:],
                        wt[:],
                        xt[:, bass.ds(col0 - poff, PSUM_CHUNK)],
                        start=True,
                        stop=True,
                    )
                    # PSUM has no DMA route: drain via both copy engines —
                    # early chunks on DVE, late on ACT, so the ACT-queue
                    # out-DMA below follows its inputs mostly in program
                    # order instead of a cross-engine wait. 5/3 split: ACT
                    # also pays ~1.2 us/batch of out-DMA descriptor-gen.
                    cols = bass.ds(col0, PSUM_CHUNK)
                    if j < 5:
                        nc.vector.tensor_copy(yt[:, cols], pt[:])
                    else:
                        nc.scalar.copy(yt[:, cols], pt[:])
                # One 1 MB out-DMA per batch row on the scalar engine's
                # HWDGE queue so pending outputs never head-of-line block
                # input loads on the sync queue.
                nc.scalar.dma_start(y[b, :, :], yt[:])
    nc.compile()
    return nc


_NC_CACHE: dict = {}


def _run(activations, inhibition_filter, use_f32r=False, io=None, **spmd_kwargs):
    act = np.ascontiguousarray(np.asarray(activations, dtype=np.float32))
    filt = np.asarray(inhibition_filter, dtype=np.float32)
    B, C, H, W = act.shape
    P = H * W
    assert B % N_CORES == 0
    b_per_core = B // N_CORES
    if io is None:
        io = "f32r" if use_f32r else "bf16"

    lhsT = _inverse_circulant_lhsT(filt, C)
    key = (b_per_core, C, P, io)
    nc = _NC_CACHE.get(key)
    if nc is None:
        nc = _NC_CACHE[key] = build_nc(b_per_core, C, P, io=io)

    if io == "bf16":
        act = act.astype(ml_dtypes.bfloat16)
        lhsT = lhsT.astype(ml_dtypes.bfloat16)
    xs = act.reshape(N_CORES, b_per_core, C, P)
    in_maps = [{"x": xs[i], "w": lhsT} for i in range(N_CORES)]
    res = run_bass_kernel_spmd(nc, in_maps, core_ids=list(range(N_CORES)), **spmd_kwargs)
    out = np.stack([res.results[i]["y"] for i in range(N_CORES)], axis=0)
    return out.reshape(B, C, H, W).astype(np.float32), res


def kernel(activations: np.ndarray, inhibition_filter: np.ndarray) -> np.ndarray:
    out, _ = _run(activations, inhibition_filter)
    return out



# revision 7
# speedup vs baseline: 1.5345x; 1.0356x over previous
"""ConvergedInhibition forward on 8 Trainium2 NeuronCores.

The reference computes, independently for every (n, h, w) pixel, a
frequency-domain deconvolution along the channel axis C=128:

    out = ifft(fft(x, axis=C) / Fk).real

Division by Fk in frequency space is circular convolution with
g = ifft(1/Fk) (real, since delta-k is real), i.e. a fixed 128x128
circulant matrix M applied to every channel vector:

    out[n, :, h, w] = M @ x[n, :, h, w],   M[c, c'] = g[(c - c') mod C]

So the heavy work is a tiny stationary matmul swept over a 134 MB
activation tensor -> memory-bound tensor-engine kernel. The length-128
filter preprocessing (FFT of a 128-vector) is negligible and done on
host in float64.

Sharding: data-parallel over batch N=64 -> 8 batches per core, no
cross-core communication. The 2e-2 rel-err gate admits bfloat16 I/O
(~4e-3 measured), which halves HBM traffic vs fp32 — the binding
constraint: the fp32 version measured at the ~358 GB/s/core HBM
roofline (93.5 us), and per-core streaming tops out ~390 GB/s with all
8 cores active.

Per-core schedule: the host hands each core its slice pre-transposed
to a flat (C, 32768) panel so DMA granularity is free. Input streams
in 8 ascending-width pieces (small first so the first matmul starts
~4 us earlier), all resident in SBUF (no ring-reuse edges). One
standalone LDWEIGHTS loads the stationary inverse-circulant into the
PE; the 64 512-col matmuls skip the per-instruction weight reload
(562 -> ~450 ns per chunk) so the PE tracks the in-stream instead of
lagging it. PSUM drains split DVE/ACT per out-block with the ACT
chunks last, so each out-DMA follows its drains in program order on
the scalar HWDGE queue; out-blocks taper at the end to keep the final
drain-out chain off the critical path.
"""

import ml_dtypes
import numpy as np

import concourse.bass as bass
import concourse.mybir as mybir
from concourse import bacc
from concourse.bass_utils import run_bass_kernel_spmd
from concourse.tile import TileContext

N_CORES = 8
PSUM_CHUNK = 512  # fp32 elements per PSUM bank


def _inverse_circulant_lhsT(filt: np.ndarray, C: int) -> np.ndarray:
    """Build the stationary matmul operand lhsT (K x M layout).

    out[m] = sum_k M[m, k] x[k] with M[m, k] = g[(m - k) mod C], and the
    tensor engine computes lhsT.T @ rhs, so lhsT[k, m] = g[(m - k) mod C].
    """
    scope = filt.shape[-1]
    pad_left = (C - scope) // 2
    k = np.zeros(C, dtype=np.float64)
    k[pad_left : pad_left + scope] = filt.reshape(-1).astype(np.float64)
    k = np.roll(k, C // 2 + 1)
    delta = np.zeros(C, dtype=np.float64)
    delta[0] = 1.0
    g = np.fft.ifft(1.0 / np.fft.fft(delta - k)).real
    j = np.arange(C)
    return g[(j[None, :] - j[:, None]) % C].astype(np.float32)


def build_nc(C: int, M: int, io: str = "bf16") -> bacc.Bacc:
    mm_dt = {
        "bf16": mybir.dt.bfloat16,
        "f32": mybir.dt.float32,
        "f32r": mybir.dt.float32r,
    }[io]
    out_dt = mybir.dt.bfloat16 if io == "bf16" else mybir.dt.float32
    nc = bacc.Bacc("TRN2", target_bir_lowering=False, debug=False)
    x = nc.dram_tensor("x", [C, M], mm_dt, kind="ExternalInput")
    w = nc.dram_tensor("w", [C, C], mm_dt, kind="ExternalInput")
    y = nc.dram_tensor("y", [C, M], out_dt, kind="ExternalOutput")

    cw = PSUM_CHUNK
    # Ascending-width input pieces: the first matmul waits only on 128 KB.
    in_widths = [cw, cw, 2 * cw, 4 * cw, 8 * cw, 16 * cw, 16 * cw, 16 * cw]
    assert sum(in_widths) == M
    # Descending-width output blocks: the last drain->out chain is short.
    out_widths = [8 * cw] * 7 + [4 * cw, 2 * cw, cw, cw]
    assert sum(out_widths) == M

    with TileContext(nc) as tc:
        with (
            tc.tile_pool(name="wp", bufs=1) as wp,
            tc.tile_pool(name="xp", bufs=1) as xp,
            tc.tile_pool(name="yp", bufs=1) as yp,
            tc.tile_pool(name="pp", bufs=8, space="PSUM") as pp,
        ):
            wt = wp.tile([C, C], mm_dt)
            nc.sync.dma_start(wt[:], w[:, :])
            pieces = []
            off = 0
            for i, pw in enumerate(in_widths):
                t = xp.tile([C, pw], mm_dt, tag=f"x{i}", bufs=1)
                nc.sync.dma_start(t[:], x[:, bass.ds(off, pw)])
                pieces.append((t, off, pw))
                off += pw

            if io == "bf16":
                nc.tensor.ldweights(wt[:])
            yoff = 0
            for i, ow in enumerate(out_widths):
                yt = yp.tile([C, ow], out_dt, tag=f"y{i}", bufs=1)
                n_ch = ow // cw
                for j in range(n_ch):
                    col0 = yoff + j * cw
                    xt, poff, pw = next(
                        p for p in pieces if p[1] <= col0 < p[1] + p[2]
                    )
                    pt = pp.tile([C, cw], mybir.dt.float32)
                    mm = nc.tensor.matmul(
                        pt[:],
                        wt[:],
                        xt[:, bass.ds(col0 - poff, cw)],
                        start=True,
                        stop=True,
                    )
                    if io == "bf16":
                        # Stationary weights never change: one LDWEIGHTS
                        # above, every matmul skips the reload. (fp32/f32r
                        # can't: walrus miscompiles non-self-loading 4-byte
                        # matmuls.)
                        mm.ins.ldweights = False
                    # PSUM has no DMA route: drain via both copy engines —
                    # early chunks on DVE, late on ACT, so the ACT-queue
                    # out-DMA below follows its drains in program order.
                    cols = bass.ds(j * cw, cw)
                    if j < (n_ch + 1) // 2 and n_ch > 1:
                        nc.vector.tensor_copy(yt[:, cols], pt[:])
                    else:
                        nc.scalar.copy(yt[:, cols], pt[:])
                nc.scalar.dma_start(y[:, bass.ds(yoff, ow)], yt[:])
                yoff += ow
    nc.compile()
    return nc


_NC_CACHE: dict = {}


def _run(activations, inhibition_filter, use_f32r=False, io=None, **spmd_kwargs):
    act = np.ascontiguousarray(np.asarray(activations, dtype=np.float32))
    filt = np.asarray(inhibition_filter, dtype=np.float32)
    B, C, H, W = act.shape
    P = H * W
    assert B % N_CORES == 0
    b_per_core = B // N_CORES
    M = b_per_core * P
    if io is None:
        io = "f32r" if use_f32r else "bf16"

    lhsT = _inverse_circulant_lhsT(filt, C)
    key = (C, M, io)
    nc = _NC_CACHE.get(key)
    if nc is None:
        nc = _NC_CACHE[key] = build_nc(C, M, io=io)

    in_dt = ml_dtypes.bfloat16 if io == "bf16" else np.float32
    # (N_CORES, b, C, P) -> per-core flat (C, b*P) panels
    xs = act.reshape(N_CORES, b_per_core, C, P).transpose(0, 2, 1, 3)
    xs = np.ascontiguousarray(xs.reshape(N_CORES, C, M), dtype=in_dt)
    lhsT = lhsT.astype(in_dt)
    in_maps = [{"x": xs[i], "w": lhsT} for i in range(N_CORES)]
    res = run_bass_kernel_spmd(nc, in_maps, core_ids=list(range(N_CORES)), **spmd_kwargs)
    out = np.stack([res.results[i]["y"] for i in range(N_CORES)], axis=0)
    out = out.reshape(N_CORES, C, b_per_core, P).transpose(0, 2, 1, 3)
    return np.ascontiguousarray(out.reshape(B, C, H, W), dtype=np.float32), res


def kernel(activations: np.ndarray, inhibition_filter: np.ndarray) -> np.ndarray:
    out, _ = _run(activations, inhibition_filter)
    return out
